# revision 20
# baseline (speedup 1.0000x reference)
"""HeteroSAGEConv Trainium2 kernel (8 NeuronCores, Bass/Tile).

Strategy (graph/data parallel, per sharding hint):
  - Destination-node rows are partitioned contiguously across the 8 cores
    (6250 user rows + 6250 item rows per core). Edges are routed host-side
    to the core owning their target node and sorted by destination block,
    so the scatter-mean is core-local. The small 128x128 per-relation
    weights are replicated.
  - Aggregation commutes with the linear projections, so each core first
    aggregates RAW source features (segment-mean) and only then applies
    the projections:  out_tgt = x_tgt @ w_tgt + segmean(x_src) @ w_src.
  - All feature data moves as bf16 (tables, messages, selectors, weights);
    accumulation stays fp32 in PSUM. This halves the dominant HBM gather
    traffic and runs the PE scatter matmuls at full bf16 rate.
  - Per 128-edge tile, source rows are fetched with dma_gather (256B bf16
    rows). int16 gather indices only reach 32767, so each source table is
    addressed via an even/odd row split (elem_step=256 elems, odd rows via
    +128 element base offset; idx = src//2).
  - The segment-sum is a tensor-engine matmul: aggT[f, d] += msgs[e, f]^T
    @ sel[e, d], where sel is the one-hot (dst match) selector pre-scaled
    by 1/deg(dst), built bf16 on the vector engine (4x DVE mode) from an
    iota row compared against per-edge dst metadata. PSUM accumulates
    across a block's edge tiles; projections then run as bf16 matmuls; the
    scalar engine does PSUM->SBUF moves and the fused relu(x*scale)
    epilogue.

Self-contained: only numpy + the installed concourse (Bass) package.
"""
import numpy as np
import ml_dtypes

import concourse.bacc as bacc
import concourse.mybir as mybir
import concourse.tile as tile
from concourse.bass_utils import run_bass_kernel_spmd

F32 = mybir.dt.float32
BF16 = mybir.dt.bfloat16
I16 = mybir.dt.int16
NP_BF16 = ml_dtypes.bfloat16

N = 50000          # nodes per type
D = 128            # feature dim
N_CORES = 8
RPC = N // N_CORES  # dst rows per core (6250)
W = 250            # dst-block width (RPC = NB * W)
NB = RPC // W      # blocks per core (25)
PW = 125           # projection half-block width (PSUM partition limit)
TILE = 128         # edges per matmul tile
CHUNK_TILES = 32   # tiles per dma_gather instruction

RELS = ("rated", "follows", "rates")  # user-dst, user-dst, item-dst

_cache = {}


# ----------------------------------------------------------------- host prep

def _pack_idx(stream_idx):
    """int16 idx stream (len = T*128) -> [128, T*8] dma_gather layout:
    tile[p, s] = idx[s*16 + p % 16]."""
    s16 = stream_idx.reshape(-1, 16).T.astype(np.int16)   # [16, S]
    return np.ascontiguousarray(np.tile(s16, (8, 1)))     # [128, S]


def _route_relation(src, dst, inv_all):
    """Sort/pad one relation's edges into per-core even/odd block streams.

    Returns (profile, per_core) where
      profile = int array [NB, 2] - tiles per (local block, half), shared
                by all cores (max over cores, so shapes are uniform);
      per_core[c] = dict with idx_ev, idx_od ([128, Th*8] int16) and
                meta ([128, Ttot, 2] f32: dst_local + j*W offset, inv).
    """
    half = (src & 1).astype(np.int64)
    gblk = dst // W                       # global block 0..399
    key = gblk * 2 + half
    order = np.argsort(key, kind="stable")
    s_src, s_dst, s_key = src[order], dst[order], key[order]
    inv_e = inv_all[s_dst]
    counts = np.bincount(key, minlength=N_CORES * NB * 2).reshape(-1, 2)
    flat = counts.reshape(-1)
    st = np.concatenate([[0], np.cumsum(flat)[:-1]]).reshape(-1, 2)
    tiles_needed = -(-counts // TILE)     # ceil
    prof = tiles_needed.reshape(N_CORES, NB, 2).max(axis=0)  # [NB, 2]

    per_core = []
    for c in range(N_CORES):
        idx_h, dloc_h, inv_h = [], [], []
        for h in range(2):
            T_tot = int(prof[:, h].sum())
            gi = np.zeros(T_tot * TILE, np.int64)
            dl = np.zeros(T_tot * TILE, np.float32)
            iv = np.zeros(T_tot * TILE, np.float32)
            off = 0
            for b in range(NB):
                j = c * NB + b
                n = int(counts[j, h])
                a = int(st[j, h])
                cap = int(prof[b, h]) * TILE
                gi[off:off + n] = s_src[a:a + n] >> 1
                dl[off:off + n] = s_dst[a:a + n] - (c * RPC + b * W)
                iv[off:off + n] = inv_e[a:a + n]
                # padding slots: idx 0, dloc stays 0 but inv 0 -> no effect
                off += cap
            idx_h.append(_pack_idx(gi))
            dloc_h.append(dl)
            inv_h.append(iv)
        # meta in consumption order: per block, ev tiles then od tiles
        T_rel = int(prof.sum())
        meta = np.zeros((128, T_rel, 2), np.float32)
        mp = 0
        cur = [0, 0]
        for b in range(NB):
            for h in range(2):
                for j in range(int(prof[b, h])):
                    t = cur[h] + j
                    sl = slice(t * TILE, (t + 1) * TILE)
                    meta[:, mp, 0] = dloc_h[h][sl]
                    meta[:, mp, 1] = inv_h[h][sl]
                    mp += 1
                cur[h] += int(prof[b, h])
        per_core.append({"idx_ev": idx_h[0], "idx_od": idx_h[1], "meta": meta})
    return prof, per_core


# ------------------------------------------------------------- device build

def _build(profiles, repeat=1, chunk_tiles=None, no_gather=False):
    """profiles: dict rel -> int array [NB, 2]. Returns compiled nc.

    repeat > 1 unrolls the whole body N times inside the NEFF (identical,
    idempotent executions) - used by the benchmark to difference out the
    fixed per-call dispatch cost and time the kernel itself on HW.

    num_swdge_queues=4: each SWDGE queue feeds its own DMA channel, so
    cycling the gather chunks across all 4 queues runs 4 gather streams
    concurrently instead of serializing on one DMA engine.
    """
    CT = chunk_tiles or CHUNK_TILES
    nc = bacc.Bacc(None, target_bir_lowering=False, num_swdge_queues=4)

    # gather tables: even/odd node rows stored contiguously ([N/2, D] each)
    # so each gathered row is a dense 256B record (no stride gaps - a strided
    # table costs 2x fetch amplification on the random-read path)
    d_tab = {}
    for tn in ("xu_ev", "xu_od", "xi_ev", "xi_od"):
        d_tab[tn] = nc.dram_tensor(tn, [N // 2, D], BF16,
                                   kind="ExternalInput")
    d_xTu = nc.dram_tensor("xTu", [D, RPC], BF16, kind="ExternalInput")
    d_xTi = nc.dram_tensor("xTi", [D, RPC], BF16, kind="ExternalInput")
    d_w = {}
    for name in ("w_user_tgt", "w_item_tgt", "w_rated_src", "w_follows_src",
                 "w_rates_src"):
        d_w[name] = nc.dram_tensor(name, [D, D], BF16, kind="ExternalInput")
    d_idx, d_meta = {}, {}
    for rel in RELS:
        prof = profiles[rel]
        for h, hn in enumerate(("ev", "od")):
            Th = int(prof[:, h].sum())
            d_idx[rel, h] = nc.dram_tensor(
                f"idx_{rel}_{hn}", [128, Th * 8], I16, kind="ExternalInput")
        d_meta[rel] = nc.dram_tensor(
            f"meta_{rel}", [128, int(prof.sum()), 2], F32,
            kind="ExternalInput")
    d_ou = nc.dram_tensor("out_user", [RPC, D], F32, kind="ExternalOutput")
    d_oi = nc.dram_tensor("out_item", [RPC, D], F32, kind="ExternalOutput")

    tabs = {"rated": (d_tab["xi_ev"][:], d_tab["xi_od"][:]),
            "follows": (d_tab["xu_ev"][:], d_tab["xu_od"][:]),
            "rates": (d_tab["xu_ev"][:], d_tab["xu_od"][:])}

    with tile.TileContext(nc) as tc:
        with (
            tc.tile_pool(name="const", bufs=1) as constp,
            tc.tile_pool(name="meta", bufs=2) as metap,
            tc.tile_pool(name="idx", bufs=2) as idxp,
            tc.tile_pool(name="msg", bufs=2) as msgp,
            tc.tile_pool(name="sel", bufs=6) as selp,
            tc.tile_pool(name="sb", bufs=3) as sbp,
            tc.tile_pool(name="psA", bufs=3, space="PSUM") as psA,
            tc.tile_pool(name="psO", bufs=2, space="PSUM") as psO,
        ):
          for _rep in range(repeat):
            iota = constp.tile([128, W], BF16)
            nc.gpsimd.iota(iota[:], pattern=[[1, W]], base=0,
                           channel_multiplier=0,
                           allow_small_or_imprecise_dtypes=True)
            w_t = {}
            for name, dt_ in d_w.items():
                w_t[name] = constp.tile([D, D], BF16, tag=name, name=name)
                nc.sync.dma_start(w_t[name][:], dt_[:])
            xTu_t = constp.tile([D, RPC], BF16, tag="xTu", name="xTu")
            nc.sync.dma_start(xTu_t[:], d_xTu[:])
            xTi_t = constp.tile([D, RPC], BF16, tag="xTi", name="xTi")
            nc.sync.dma_start(xTi_t[:], d_xTi[:])
            arena = constp.tile([128, RPC], BF16, tag="arena", name="arena")
            fake_msg = None
            if no_gather:  # timing diagnostic: zero msgs, no gather DMA
                fake_msg = constp.tile([128, CT, D], BF16, tag="fkm",
                                       name="fkm")
                nc.vector.memset(fake_msg[:], 0.0)

            qctr = [0]

            def sweep(rel, consume):
                prof = profiles[rel]
                tab = tabs[rel]
                T_half = [int(prof[:, h].sum()) for h in range(2)]
                meta_t = metap.tile([128, int(prof.sum()), 2], F32, tag="meta")
                nc.sync.dma_start(meta_t[:], d_meta[rel][:])
                chunk_tiles = [[], []]

                def ensure_chunk(h, c):
                    while len(chunk_tiles[h]) <= c:
                        if no_gather:
                            chunk_tiles[h].append(fake_msg)
                            continue
                        cc = len(chunk_tiles[h])
                        t0 = cc * CT
                        tcnt = min(CT, T_half[h] - t0)
                        it = idxp.tile([128, CT * 8], I16,
                                       tag=f"idx_{rel}{h}",
                                       name=f"idx_{rel}{h}")
                        nc.sync.dma_start(
                            it[:, :tcnt * 8],
                            d_idx[rel, h][:, t0 * 8:(t0 + tcnt) * 8])
                        m = msgp.tile([128, CT, D], BF16,
                                      tag=f"msg_{rel}{h}",
                                      name=f"msg_{rel}{h}")
                        nc.gpsimd.dma_gather(
                            m[:, :tcnt, :], tab[h], it[:, :tcnt * 8],
                            tcnt * TILE, tcnt * TILE,
                            elem_size=D,
                            single_packet=False,
                            queue_num=qctr[0] % 4)
                        qctr[0] += 1
                        chunk_tiles[h].append(m)

                mp = 0
                cur = [0, 0]
                for b in range(NB):
                    nt = int(prof[b, 0] + prof[b, 1])
                    aggT = psA.tile([128, W], F32, tag="agg", name="agg") if nt else None
                    k = 0
                    for h in range(2):
                        Tb = int(prof[b, h])
                        if Tb == 0:
                            continue
                        for j in range(Tb):
                            t = cur[h] + j
                            ensure_chunk(h, t // CT)
                            mtile = chunk_tiles[h][t // CT][
                                :, t % CT, :]
                            sel = selp.tile([128, W], BF16, tag="sel", name="sel")
                            nc.vector.tensor_scalar(
                                sel[:], iota[:],
                                meta_t[:, mp, 0:1], meta_t[:, mp, 1:2],
                                op0=mybir.AluOpType.is_equal,
                                op1=mybir.AluOpType.mult)
                            nc.tensor.matmul(
                                aggT[:], mtile, sel[:],
                                start=(k == 0), stop=(k == nt - 1))
                            k += 1
                            mp += 1
                        cur[h] += Tb
                    consume(b, aggT)

            # phase U1: rated -> arena
            def consume_rated(b, aggT):
                dst = arena[:, b * W:(b + 1) * W]
                if aggT is None:
                    nc.vector.memset(dst, 0.0)
                else:
                    nc.scalar.activation(dst, aggT[:],
                                         mybir.ActivationFunctionType.Copy)

            sweep("rated", consume_rated)

            # phase U2: follows + user projections/epilogue
            def consume_follows(b, aggT):
                aggf = None
                if aggT is not None:
                    aggf = sbp.tile([128, W], BF16, tag="aggf", name="aggf")
                    nc.scalar.activation(aggf[:], aggT[:],
                                         mybir.ActivationFunctionType.Copy)
                for j in range(W // PW):
                    r0 = b * W + j * PW
                    outp = psO.tile([PW, D], F32, tag="out", name="out")
                    nc.tensor.matmul(outp[:], xTu_t[:, r0:r0 + PW],
                                     w_t["w_user_tgt"][:], start=True,
                                     stop=False)
                    nc.tensor.matmul(outp[:], arena[:, r0:r0 + PW],
                                     w_t["w_rated_src"][:], start=False,
                                     stop=aggf is None)
                    if aggf is not None:
                        nc.tensor.matmul(outp[:],
                                         aggf[:, j * PW:(j + 1) * PW],
                                         w_t["w_follows_src"][:], start=False,
                                         stop=True)
                    outs = sbp.tile([PW, D], F32, tag="outs", name="outs")
                    nc.scalar.activation(outs[:], outp[:],
                                         mybir.ActivationFunctionType.Relu,
                                         scale=0.5)
                    nc.sync.dma_start(d_ou[r0:r0 + PW, :], outs[:])

            sweep("follows", consume_follows)

            # phase I1: rates + item projections/epilogue
            def consume_rates(b, aggT):
                aggf = None
                if aggT is not None:
                    aggf = sbp.tile([128, W], BF16, tag="aggf", name="aggf")
                    nc.scalar.activation(aggf[:], aggT[:],
                                         mybir.ActivationFunctionType.Copy)
                for j in range(W // PW):
                    r0 = b * W + j * PW
                    outp = psO.tile([PW, D], F32, tag="out", name="out")
                    nc.tensor.matmul(outp[:], xTi_t[:, r0:r0 + PW],
                                     w_t["w_item_tgt"][:], start=True,
                                     stop=aggf is None)
                    if aggf is not None:
                        nc.tensor.matmul(outp[:],
                                         aggf[:, j * PW:(j + 1) * PW],
                                         w_t["w_rates_src"][:],
                                         start=False, stop=True)
                    outs = sbp.tile([PW, D], F32, tag="outs", name="outs")
                    nc.scalar.activation(outs[:], outp[:],
                                         mybir.ActivationFunctionType.Relu)
                    nc.sync.dma_start(d_oi[r0:r0 + PW, :], outs[:])

            sweep("rates", consume_rates)

    nc.compile()
    return nc


# ------------------------------------------------------------------- driver

def _bf16(a):
    return np.ascontiguousarray(np.asarray(a, np.float32).astype(NP_BF16))


def kernel(x_user, x_item, w_rates_src, w_rates_tgt, w_rated_src,
           w_rated_tgt, w_follows_src, w_follows_tgt, edge_rates,
           edge_rated, edge_follows, _profile=False):
    x_user = np.ascontiguousarray(np.asarray(x_user, np.float32))
    x_item = np.ascontiguousarray(np.asarray(x_item, np.float32))
    ws = {k: np.asarray(v, np.float32) for k, v in {
        "w_rates_src": w_rates_src, "w_rates_tgt": w_rates_tgt,
        "w_rated_src": w_rated_src, "w_rated_tgt": w_rated_tgt,
        "w_follows_src": w_follows_src, "w_follows_tgt": w_follows_tgt,
    }.items()}
    e_rates = np.asarray(edge_rates, np.int64)
    e_rated = np.asarray(edge_rated, np.int64)
    e_follows = np.asarray(edge_follows, np.int64)

    # per-dst mean denominators (deg clamped to 1)
    inv_item = 1.0 / np.maximum(
        np.bincount(e_rates[1], minlength=N), 1.0).astype(np.float32)
    inv_user_rated = 1.0 / np.maximum(
        np.bincount(e_rated[1], minlength=N), 1.0).astype(np.float32)
    inv_user_fol = 1.0 / np.maximum(
        np.bincount(e_follows[1], minlength=N), 1.0).astype(np.float32)

    routes, profiles = {}, {}
    profiles["rated"], routes["rated"] = _route_relation(
        e_rated[0], e_rated[1], inv_user_rated)
    profiles["follows"], routes["follows"] = _route_relation(
        e_follows[0], e_follows[1], inv_user_fol)
    profiles["rates"], routes["rates"] = _route_relation(
        e_rates[0], e_rates[1], inv_item)

    key = tuple(profiles[r].tobytes() for r in RELS)
    if key not in _cache:
        _cache.clear()
        _cache[key] = _build(profiles)
    nc = _cache[key]

    xu_ev, xu_od = _bf16(x_user[0::2]), _bf16(x_user[1::2])
    xi_ev, xi_od = _bf16(x_item[0::2]), _bf16(x_item[1::2])
    w_user_tgt = _bf16(ws["w_rated_tgt"] + ws["w_follows_tgt"])
    in_maps = []
    for c in range(N_CORES):
        rows = slice(c * RPC, (c + 1) * RPC)
        im = {
            "xu_ev": xu_ev, "xu_od": xu_od, "xi_ev": xi_ev, "xi_od": xi_od,
            "xTu": _bf16(x_user[rows].T),
            "xTi": _bf16(x_item[rows].T),
            "w_user_tgt": w_user_tgt, "w_item_tgt": _bf16(ws["w_rates_tgt"]),
            "w_rated_src": _bf16(ws["w_rated_src"]),
            "w_follows_src": _bf16(ws["w_follows_src"]),
            "w_rates_src": _bf16(ws["w_rates_src"]),
        }
        for rel in RELS:
            im[f"idx_{rel}_ev"] = routes[rel][c]["idx_ev"]
            im[f"idx_{rel}_od"] = routes[rel][c]["idx_od"]
            im[f"meta_{rel}"] = routes[rel][c]["meta"]
        in_maps.append(im)

    kernel._last_nc = nc
    kernel._last_in_maps = in_maps
    res = run_bass_kernel_spmd(nc, in_maps, core_ids=list(range(N_CORES)),
                               trace=_profile)
    out_user = np.concatenate([res.results[c]["out_user"]
                               for c in range(N_CORES)], axis=0)
    out_item = np.concatenate([res.results[c]["out_item"]
                               for c in range(N_CORES)], axis=0)
    if _profile:
        kernel._last_exec_ns = res.exec_time_ns
        kernel._last_profile = res.profile_json
    return out_user, out_item
